# revision 8
# baseline (speedup 1.0000x reference)
"""GroupLinear (MoE routing) Trainium2 kernel.

Problem: x [8,2048,1024] f32, group_by [8,2048] int32 in [0,8),
W [8, 1024*1024] f32 (row g -> (dout,din) weight), b [8,1024] f32.
out[b,s,:] = W[g].reshape(1024,1024) @ x[b,s,:] + b[g],  g = group_by[b,s].

Strategy: expert-parallel over 8 NeuronCores. Core g gets every token
routed to group g (host-side dispatch), its own weight (pre-transposed to
[din, dout] so the contraction dim lands on SBUF partitions), and bias.
On device each core runs a single dense GEMM  Y[C,1024] = Xg @ Wg^T + bg
with fp32r (full-rate fp32) matmuls: stationary = X^T tile [128d,128t],
moving = W^T tile [128d,512o], PSUM accumulates over the 8 k-chunks.
Host scatters the per-core results back to token order.
"""

import numpy as np
from contextlib import ExitStack

import concourse.bass as bass
import concourse.mybir as mybir
import concourse.tile as tile
from concourse import bacc
from concourse.bass_utils import run_bass_kernel_spmd

B, S, DIN, DOUT, G = 8, 2048, 1024, 1024, 8
P = 128
C = 2176          # per-core token capacity (17 * 128); seed-0 max count is 2088
KC = DIN // P     # 8 contraction chunks
TB = C // P       # 17 token blocks
OH = DOUT // 512  # 2 moving halves (fp32 moving max = 512)

_cache = {}


def _emit(ctx, tc, y, xt, wt, bias, reps=1):
    nc = tc.nc
    f32 = mybir.dt.float32
    f32r = mybir.dt.float32r

    singles = ctx.enter_context(tc.tile_pool(name="singles", bufs=1))
    xpool = ctx.enter_context(tc.tile_pool(name="xpool", bufs=8))
    opool = ctx.enter_context(tc.tile_pool(name="opool", bufs=4))
    psum = ctx.enter_context(tc.tile_pool(name="psum", bufs=8, space="PSUM"))

    xt_r = xt.rearrange("(k p) t -> p k t", p=P)
    PH0 = 4  # t-blocks covered by the k-outer warmup phase (PH0*OH psum banks)

    def load_xt(tb):
        xt_tile = xpool.tile([P, KC, P], f32r, name="xt_tile", tag="xt_tile")
        # ACT HWDGE ring: runs concurrently with the weight stream on SP's
        nc.scalar.dma_start(out=xt_tile, in_=xt_r[:, :, tb * P:(tb + 1) * P].bitcast(f32r))
        return xt_tile

    # Prefetch the warmup blocks ahead of the bulk weight stream.
    prefetched = {tb: load_xt(tb) for tb in range(PH0)}

    # Whole weight table resident in SBUF: [128, 8 kchunks, 1024] = 32KB/part.
    # Loaded as one DMA per k-chunk so the first matmuls only wait for chunk 0.
    wt_sb = singles.tile([P, KC, DOUT], f32r)
    wt_r = wt.rearrange("(k p) o -> p k o", p=P).bitcast(f32r)
    for k in range(KC):
        nc.sync.dma_start(out=wt_sb[:, k, :], in_=wt_r[:, k, :])
    bias_sb = singles.tile([P, DOUT], f32)
    nc.sync.dma_start(out=bias_sb, in_=bias)

    def emit_out(ps, tb, oh):
        ot = opool.tile([P, 512], f32, name="ot", tag="ot")
        nc.vector.tensor_add(out=ot, in0=ps, in1=bias_sb[:, oh * 512:(oh + 1) * 512])
        nc.gpsimd.dma_start(out=y[tb * P:(tb + 1) * P, oh * 512:(oh + 1) * 512], in_=ot)

    for _rep in range(reps):
        if _rep == 0:
            # Phase 0: k-outer across PH0 blocks x both halves so the PE
            # consumes each weight chunk the moment its DMA lands instead of
            # stalling a full psum group on the whole 4MB weight stream.
            tiles0 = [prefetched.pop(tb) for tb in range(PH0)]
            ps0 = [psum.tile([P, 512], f32, name="ps", tag="ps")
                   for _ in range(PH0 * OH)]
            for k in range(KC):
                for i in range(PH0 * OH):
                    tb, oh = divmod(i, OH)
                    nc.tensor.matmul(
                        ps0[i],
                        lhsT=tiles0[tb][:, k, :],
                        rhs=wt_sb[:, k, oh * 512:(oh + 1) * 512],
                        start=(k == 0),
                        stop=(k == KC - 1),
                    )
            for i in range(PH0 * OH):
                tb, oh = divmod(i, OH)
                emit_out(ps0[i], tb, oh)
        start_tb = PH0 if _rep == 0 else 0
        for tb in range(start_tb, TB):
            xt_tile = load_xt(tb)
            for oh in range(OH):
                ps = psum.tile([P, 512], f32, name="ps", tag="ps")
                for k in range(KC):
                    nc.tensor.matmul(
                        ps,
                        lhsT=xt_tile[:, k, :],
                        rhs=wt_sb[:, k, oh * 512:(oh + 1) * 512],
                        start=(k == 0),
                        stop=(k == KC - 1),
                    )
                emit_out(ps, tb, oh)


def _build(reps=1):
    key = ("nc", reps)
    if key in _cache:
        return _cache[key]
    nc = bacc.Bacc("TRN2", target_bir_lowering=False, debug=False,
                   enable_asserts=False, num_devices=G)
    f32 = mybir.dt.float32
    xt = nc.dram_tensor("xt", [DIN, C], f32, kind="ExternalInput").ap()
    wt = nc.dram_tensor("wt", [DIN, DOUT], f32, kind="ExternalInput").ap()
    bias = nc.dram_tensor("bias", [P, DOUT], f32, kind="ExternalInput").ap()
    y = nc.dram_tensor("y", [C, DOUT], f32, kind="ExternalOutput").ap()
    with tile.TileContext(nc) as tc, ExitStack() as ctx:
        _emit(ctx, tc, y, xt, wt, bias, reps=reps)
    nc.compile()
    _cache[key] = nc
    return nc


def _prep_inputs(x, group_by, W, b):
    x_flat = np.ascontiguousarray(np.asarray(x, dtype=np.float32)).reshape(B * S, DIN)
    gb = np.asarray(group_by).reshape(B * S)
    W = np.asarray(W, dtype=np.float32)
    b = np.asarray(b, dtype=np.float32)

    idxs, in_maps, spill = [], [], []
    for g in range(G):
        idx = np.nonzero(gb == g)[0]
        n = min(len(idx), C)
        xt = np.zeros((DIN, C), dtype=np.float32)
        xt[:, :n] = x_flat[idx[:n]].T
        wt = np.ascontiguousarray(W[g].reshape(DOUT, DIN).T)
        bias = np.ascontiguousarray(np.broadcast_to(b[g], (P, DOUT)))
        in_maps.append({"xt": xt, "wt": wt, "bias": bias})
        idxs.append(idx)
        if len(idx) > C:
            spill.append(g)
    return x_flat, idxs, in_maps, spill, W, b


def kernel(x, group_by, W, b, _trace=False):
    nc = _build()
    x_flat, idxs, in_maps, spill, W, b = _prep_inputs(x, group_by, W, b)
    res = run_bass_kernel_spmd(nc, in_maps, list(range(G)), trace=_trace)

    out_flat = np.empty((B * S, DOUT), dtype=np.float32)
    for g in range(G):
        idx = idxs[g]
        n = min(len(idx), C)
        out_flat[idx[:n]] = res.results[g]["y"][:n]
        if len(idx) > C:  # capacity spill: finish the stragglers on host
            extra = idx[C:]
            out_flat[extra] = x_flat[extra] @ W[g].reshape(DOUT, DIN).T + b[g]
    out = out_flat.reshape(B, S, DOUT)
    if _trace:
        return out, res
    return out


# revision 9
# speedup vs baseline: 1.0842x; 1.0842x over previous
"""GroupLinear (MoE routing) Trainium2 kernel.

Problem: x [8,2048,1024] f32, group_by [8,2048] int32 in [0,8),
W [8, 1024*1024] f32 (row g -> (dout,din) weight), b [8,1024] f32.
out[b,s,:] = W[g].reshape(1024,1024) @ x[b,s,:] + b[g],  g = group_by[b,s].

Strategy: expert-parallel over 8 NeuronCores. Core g gets every token
routed to group g (host-side dispatch), its own weight (pre-transposed to
[din, dout] so the contraction dim lands on SBUF partitions), and bias.
On device each core runs a single dense GEMM  Y[C,1024] = Xg @ Wg^T + bg:
stationary = X^T tile [128d,128t], moving = W^T tile [128d,512o], PSUM
accumulates over the 8 k-chunks. Tokens beyond the C capacity (none at
seed-0 counts for C>=2088) are finished on the host.
Host scatters the per-core results back to token order.
"""

import numpy as np
from contextlib import ExitStack

import concourse.bass as bass
import concourse.mybir as mybir
import concourse.tile as tile
from concourse import bacc
from concourse.bass_utils import run_bass_kernel_spmd

B, S, DIN, DOUT, G = 8, 2048, 1024, 1024, 8
P = 128
KC = DIN // P     # 8 contraction chunks
OH = DOUT // 512  # 2 moving halves (moving free dim caps at one PSUM bank)

# Default config (overridable per build for experiments)
C_DEFAULT = 2176          # per-core token capacity; seed-0 max count is 2088
DT_DEFAULT = "f32r"       # matmul operand dtype: f32r | f16 | bf16

_cache = {}


def _emit(ctx, tc, y, xt, wt, bias, C, mdt, reps=1):
    nc = tc.nc
    f32 = mybir.dt.float32
    TB = C // P

    singles = ctx.enter_context(tc.tile_pool(name="singles", bufs=1))
    xpool = ctx.enter_context(tc.tile_pool(name="xpool", bufs=8))
    opool = ctx.enter_context(tc.tile_pool(name="opool", bufs=4))
    psum = ctx.enter_context(tc.tile_pool(name="psum", bufs=8, space="PSUM"))

    xt_r = xt.rearrange("(k p) t -> p k t", p=P)
    PH0 = 4  # t-blocks covered by the k-outer warmup phase (PH0*OH psum banks)

    def load_xt(tb):
        xt_tile = xpool.tile([P, KC, P], mdt, name="xt_tile", tag="xt_tile")
        # ACT HWDGE ring: runs concurrently with the weight stream on SP's
        nc.scalar.dma_start(out=xt_tile, in_=xt_r[:, :, tb * P:(tb + 1) * P])
        return xt_tile

    # Prefetch the warmup blocks ahead of the bulk weight stream.
    prefetched = {tb: load_xt(tb) for tb in range(PH0)}

    # Whole weight table resident in SBUF, loaded one DMA per k-chunk so the
    # first matmuls only wait for chunk 0.
    wt_sb = singles.tile([P, KC, DOUT], mdt)
    wt_r = wt.rearrange("(k p) o -> p k o", p=P)
    for k in range(KC):
        nc.sync.dma_start(out=wt_sb[:, k, :], in_=wt_r[:, k, :])
    bias_sb = singles.tile([P, DOUT], f32)
    nc.sync.dma_start(out=bias_sb, in_=bias)

    def emit_out(ps, tb, oh):
        ot = opool.tile([P, 512], f32, name="ot", tag="ot")
        nc.vector.tensor_add(out=ot, in0=ps, in1=bias_sb[:, oh * 512:(oh + 1) * 512])
        nc.gpsimd.dma_start(out=y[tb * P:(tb + 1) * P, oh * 512:(oh + 1) * 512], in_=ot)

    def mm(ps, xt_tile, k, oh):
        nc.tensor.matmul(
            ps,
            lhsT=xt_tile[:, k, :],
            rhs=wt_sb[:, k, oh * 512:(oh + 1) * 512],
            start=(k == 0),
            stop=(k == KC - 1),
        )

    for _rep in range(reps):
        if _rep == 0:
            # Phase 0: k-outer across PH0 blocks x both halves so the PE
            # consumes each weight chunk the moment its DMA lands instead of
            # stalling a full psum group on the whole 4MB weight stream.
            tiles0 = [prefetched.pop(tb) for tb in range(PH0)]
            ps0 = [psum.tile([P, 512], f32, name="ps", tag="ps")
                   for _ in range(PH0 * OH)]
            for k in range(KC):
                for i in range(PH0 * OH):
                    tb, oh = divmod(i, OH)
                    mm(ps0[i], tiles0[tb], k, oh)
            for i in range(PH0 * OH):
                tb, oh = divmod(i, OH)
                emit_out(ps0[i], tb, oh)
        start_tb = PH0 if _rep == 0 else 0
        for tb in range(start_tb, TB):
            xt_tile = load_xt(tb)
            for oh in range(OH):
                ps = psum.tile([P, 512], f32, name="ps", tag="ps")
                for k in range(KC):
                    mm(ps, xt_tile, k, oh)
                emit_out(ps, tb, oh)


def _build(reps=1, C=C_DEFAULT, dt=DT_DEFAULT):
    key = (reps, C, dt)
    if key in _cache:
        return _cache[key]
    nc = bacc.Bacc("TRN2", target_bir_lowering=False, debug=False,
                   enable_asserts=False, num_devices=G)
    f32 = mybir.dt.float32
    mdt = {"f32r": mybir.dt.float32r, "f16": mybir.dt.float16,
           "bf16": mybir.dt.bfloat16}[dt]
    # For f32r the DRAM inputs carry the same bits as f32; declaring them
    # f32r end-to-end keeps the BIR verifier's rounding rule satisfied.
    in_dt = mdt if dt != "f32r" else mybir.dt.float32r
    xt = nc.dram_tensor("xt", [DIN, C], in_dt, kind="ExternalInput").ap()
    wt = nc.dram_tensor("wt", [DIN, DOUT], in_dt, kind="ExternalInput").ap()
    bias = nc.dram_tensor("bias", [P, DOUT], f32, kind="ExternalInput").ap()
    y = nc.dram_tensor("y", [C, DOUT], f32, kind="ExternalOutput").ap()
    with tile.TileContext(nc) as tc, ExitStack() as ctx:
        _emit(ctx, tc, y, xt, wt, bias, C, mdt, reps=reps)
    nc.compile()
    _cache[key] = nc
    return nc


def _np_dt(dt):
    return {"f32r": np.float32, "f16": np.float16, "bf16": None}[dt]


def _prep_inputs(x, group_by, W, b, C=C_DEFAULT, dt=DT_DEFAULT):
    import ml_dtypes
    np_dt = np.float32 if dt == "f32r" else (
        np.float16 if dt == "f16" else ml_dtypes.bfloat16)
    x_flat = np.ascontiguousarray(np.asarray(x, dtype=np.float32)).reshape(B * S, DIN)
    gb = np.asarray(group_by).reshape(B * S)
    W = np.asarray(W, dtype=np.float32)
    b = np.asarray(b, dtype=np.float32)

    idxs, in_maps = [], []
    for g in range(G):
        idx = np.nonzero(gb == g)[0]
        n = min(len(idx), C)
        xt = np.zeros((DIN, C), dtype=np_dt)
        xt[:, :n] = x_flat[idx[:n]].T.astype(np_dt)
        wt = np.ascontiguousarray(W[g].reshape(DOUT, DIN).T.astype(np_dt))
        bias = np.ascontiguousarray(np.broadcast_to(b[g], (P, DOUT)))
        in_maps.append({"xt": xt, "wt": wt, "bias": bias})
        idxs.append(idx)
    return x_flat, idxs, in_maps, W, b


def _scatter(results, x_flat, idxs, W, b, C=C_DEFAULT):
    out_flat = np.empty((B * S, DOUT), dtype=np.float32)
    for g in range(G):
        idx = idxs[g]
        n = min(len(idx), C)
        out_flat[idx[:n]] = results[g]["y"][:n]
        if len(idx) > C:  # capacity spill: finish the stragglers on host
            extra = idx[C:]
            out_flat[extra] = x_flat[extra] @ W[g].reshape(DOUT, DIN).T + b[g]
    return out_flat.reshape(B, S, DOUT)


def kernel(x, group_by, W, b):
    nc = _build()
    x_flat, idxs, in_maps, W, b = _prep_inputs(x, group_by, W, b)
    res = run_bass_kernel_spmd(nc, in_maps, list(range(G)))
    return _scatter(res.results, x_flat, idxs, W, b)
